# revision 8
# baseline (speedup 1.0000x reference)
"""Trainium2 Bass kernel for DualThresholdSelfregulatingIntegrate.

Computes (to within tolerance of the jax reference):
    rates  = relu(x) * DT                     # [B, T, D]
    c      = init[:, None, :] + cumsum(rates, axis=1)
    spikes = floor(c) - floor(c_prev)         # spikes in {0, 1}
    out    = spikes / DT                      # values {0, 1000}

Strategy (per core, pure data-parallel over batch; BC=2 batches/core):
  - natural-layout loads [t, d] batched 2 time-chunks per DMA; relu on ACT
  - cumsum over each 128-step time chunk via one fp32 PE matmul per
    (chunk, d-block): psum[d, t'] = sum_t rates[t, d] * U[t, t'] with U
    upper-triangular ones; sequential PSUM accumulation reproduces the
    reference's f32 rounding closely (sim: 2 wrong elements of 33.5M)
  - floors Gi = rint(pc + (S + v0 - 0.5)) as int16 in one fused op per
    chunk (DVE stt for most d-blocks, ACT activation-with-bias for the
    rest); Kahan-compensated inter-chunk carry runs on GPSIMD (Pool)
  - output bit-packed on PE: Gi (int16) -> fp16, then a powers-of-2
    block matmul accumulates P = sum_i 2^i * Gi over 8-chunk groups in
    PSUM; spike bytes emerge as P_t - P_{t-1} (diff commutes with the
    linear pack), computed by one DVE subtract of shifted psum views
    directly into a uint8 tile; DMA out is 1 KB/partition per group
    (32x less write traffic than f32); host unpacks bits -> f32
"""

import sys

sys.path.insert(0, "/opt/trn_rl_repo")

import numpy as np

import concourse.bass as bass  # noqa: F401  (registers engines)
import concourse.tile as tile
from concourse import bacc, mybir

N_CORES = 8
B, T, D = 16, 2048, 1024
BC = B // N_CORES          # batches per core
CH = 128                   # time-chunk (matmul contraction) size
NCH = T // CH              # 16 chunks per batch row
NDB = D // CH              # 8 d-blocks
GRP = 8                    # chunks per output pack group
NGR = NCH // GRP           # 2 groups per batch row
NJP = 4                    # j-pair pack matmuls per chunk
dt = mybir.dt

K1000 = float(np.float32(1.0) / np.float32(0.001))   # matches reference's /DT

_cache = {}
CFG = {
    "batch": 2,        # time-chunks per input DMA / relu op
    "actsplit": 0,     # d-blocks of the floor op done on ACT (rest on DVE)
    "gib": 3,          # Gi/Gf pool buffers
    "smb": 8,          # small-tile pool buffers
    "pcb": 2,          # cumsum PSUM buffers
    "bb": 3,           # input tile pool buffers
    "loadeng": "sync",
    "storeeng": "sync",
}


def build_nc(bench_iters=0, skip=()):
    nc = bacc.Bacc("TRN2", target_bir_lowering=False, debug=False)
    x = nc.dram_tensor("x", [BC, T, D], dt.float32, kind="ExternalInput")
    v0m = nc.dram_tensor("v0m", [BC, CH, NDB], dt.float32,
                         kind="ExternalInput")
    u = nc.dram_tensor("u", [CH, CH], dt.float32, kind="ExternalInput")
    pks = nc.dram_tensor("pks", [GRP, CH, CH], dt.float16,
                         kind="ExternalInput")
    y_pk = nc.dram_tensor("y_pk", [BC, NGR, CH, GRP * CH], dt.uint8,
                          kind="ExternalOutput")

    with tile.TileContext(nc) as tc:
        with tc.tile_pool(name="xin", bufs=CFG["bb"]) as xin_p, \
             tc.tile_pool(name="rates", bufs=CFG["bb"]) as rates_p, \
             tc.tile_pool(name="gi", bufs=CFG["gib"]) as gi_p, \
             tc.tile_pool(name="gf", bufs=CFG["gib"]) as gf_p, \
             tc.tile_pool(name="oo", bufs=2) as o_p, \
             tc.tile_pool(name="small", bufs=CFG["smb"]) as sm_p, \
             tc.tile_pool(name="consts", bufs=1) as c_p, \
             tc.tile_pool(name="pc", bufs=CFG["pcb"], space="PSUM") as pc_p, \
             tc.tile_pool(name="pp", bufs=1, space="PSUM") as pp_p:

            ut = c_p.tile([CH, CH], dt.float32, tag="ut")
            nc.sync.dma_start(ut[:], u[:])
            pkt = c_p.tile([CH, GRP * CH], dt.float16, tag="pkt")
            nc.sync.dma_start(
                pkt[:].rearrange("p (m c) -> p m c", m=GRP),
                pks[:].rearrange("m p c -> p m c"),
            )
            i2mb = c_p.tile([CH, BC * NDB], dt.float32, tag="i2mb")
            nc.sync.dma_start(
                i2mb[:].rearrange("p (b j) -> p b j", b=BC),
                v0m[:].rearrange("b p j -> p b j"),
            )
            i2m3 = i2mb[:].rearrange("p (b j) -> p b j", b=BC)

            import contextlib
            _hints = (mybir.EngineType.DVE, mybir.EngineType.Activation,
                      mybir.EngineType.PE, mybir.EngineType.SP,
                      mybir.EngineType.Pool)
            loop_cm = tc.For_i(0, bench_iters, 1, hint_engines=_hints) \
                if bench_iters else contextlib.nullcontext()
            with loop_cm:
                body(nc, tc, x, y_pk, i2m3, ut, pkt,
                     xin_p, rates_p, gi_p, gf_p, o_p, sm_p, pc_p, pp_p,
                     skip=set(skip))
    nc.compile()
    return nc


def body(nc, tc, x, y_pk, i2m3, ut, pkt,
         xin_p, rates_p, gi_p, gf_p, o_p, sm_p, pc_p, pp_p, skip=()):
    AL = mybir.AluOpType
    AF = mybir.ActivationFunctionType
    NB = CFG["batch"]
    NA = CFG["actsplit"]
    pk3 = pkt[:].rearrange("p (m c) -> p m c", m=GRP)

    for b in range(BC):
        # Kahan state (f32, [CH, NDB]) lives on Pool
        S_old = sm_p.tile([CH, NDB], dt.float32, tag="skah")
        nc.gpsimd.memset(S_old[:], 0.0)
        comp_old = sm_p.tile([CH, NDB], dt.float32, tag="ckah")
        nc.gpsimd.memset(comp_old[:], 0.0)
        carrm = i2m3[:, b, :]          # S=0: carry-minus-half = v0 - 0.5
        gf_prev3 = None
        pp_t = None

        for k in range(NCH):
            m = k % GRP
            G = k // GRP
            if k % NB == 0:
                _le = getattr(nc, CFG.get("loadeng", "sync"))
                x2 = xin_p.tile([CH, NB * D], dt.float32, tag="xk")
                _le.dma_start(
                    x2[:].rearrange("p (c d) -> p c d", c=NB),
                    x[b, k * CH:(k + NB) * CH, :].rearrange(
                        "(c p) d -> p c d", p=CH))
                if "relu" in skip:
                    r2 = x2
                else:
                    r2 = rates_p.tile([CH, NB * D], dt.float32, tag="rk")
                    nc.scalar.activation(r2[:], x2[:], AF.Relu,
                                         bias=0.0, scale=0.001)
            rk = r2[:, (k % NB) * D:(k % NB + 1) * D]

            # chunk-local cumsum: psum[d, t'] per d-block
            pck = pc_p.tile([CH, D], dt.float32, tag="pck")
            njr = 1 if "mm" in skip else NDB
            for j in range(njr):
                nc.tensor.matmul(pck[:, j * CH:(j + 1) * CH],
                                 rk[:, j * CH:(j + 1) * CH], ut[:],
                                 start=True, stop=True)
            pc3 = pck[:].rearrange("p (j t) -> p j t", j=NDB)

            # floor: Gi = rint(pc + carrm) as int16 (carrm = S + v0 - 0.5)
            gik = gi_p.tile([CH, D], dt.int16, tag="gik")
            g3 = gik[:].rearrange("p (j t) -> p j t", j=NDB)
            if "floor" not in skip:
                for j in range(NA):
                    nc.scalar.activation(g3[:, j, :], pc3[:, j, :],
                                         AF.Relu, bias=carrm[:, j:j + 1],
                                         scale=1.0)
                cb = carrm[:, NA:].unsqueeze(2).broadcast_to(
                    [CH, NDB - NA, CH])
                nc.vector.scalar_tensor_tensor(g3[:, NA:, :], pc3[:, NA:, :],
                                               1.0, cb,
                                               op0=AL.mult, op1=AL.add)

            # Kahan carry update from chunk totals pc[:, :, CH-1]
            pcol = sm_p.tile([CH, NDB], dt.float32, tag="pcol")
            nc.vector.tensor_copy(pcol[:], pc3[:, :, CH - 1])
            ykh = sm_p.tile([CH, NDB], dt.float32, tag="ykh")
            nc.gpsimd.tensor_tensor(ykh[:], pcol[:], comp_old[:],
                                    op=AL.subtract)
            S_new = sm_p.tile([CH, NDB], dt.float32, tag="skah")
            nc.gpsimd.tensor_tensor(S_new[:], S_old[:], ykh[:], op=AL.add)
            carrm_new = sm_p.tile([CH, NDB], dt.float32, tag="carrm")
            nc.gpsimd.tensor_tensor(carrm_new[:], S_new[:], i2m3[:, b, :],
                                    op=AL.add)
            dkh = sm_p.tile([CH, NDB], dt.float32, tag="dkh")
            nc.gpsimd.tensor_tensor(dkh[:], S_new[:], S_old[:],
                                    op=AL.subtract)
            comp_new = sm_p.tile([CH, NDB], dt.float32, tag="ckah")
            nc.gpsimd.tensor_tensor(comp_new[:], dkh[:], ykh[:],
                                    op=AL.subtract)
            S_old, comp_old, carrm = S_new, comp_new, carrm_new[:]

            if "out" in skip:
                continue

            # int16 -> fp16 with [pad, boundary, data...] layout per d-block
            # (pad col keeps the data write 4B-aligned for DVE 4x mode)
            gfk = gf_p.tile([CH, NDB * (CH + 2)], dt.float16, tag="gfk")
            gf3 = gfk[:].rearrange("p (j t) -> p j t", j=NDB)
            if m == 0 and G == 0:
                nc.gpsimd.memset(gf3[:, :, 1], 0.0)   # floor(v0) == 0
            else:
                nc.gpsimd.tensor_copy(gf3[:, :, 1], gf_prev3[:, :, CH + 1])
            nc.vector.tensor_copy(gf3[:, :, 2:CH + 2], g3)
            gf_prev3 = gf3

            # pack matmuls: P[16m+g, (j,t)] += sum_i 2^i * Gi[8g+i, (j,t)]
            if m == 0:
                pp_t = [pp_p.tile([CH, 2 * (CH + 1)], dt.float32,
                                  name=f"pp{jp}", tag=f"pp{jp}")
                        for jp in range(NJP)]
            for jp in range(NJP):
                nc.tensor.matmul(
                    pp_t[jp][:],
                    pk3[:, m, :],
                    gf3[:, 2 * jp:2 * jp + 2, 1:CH + 2],
                    start=(m == 0), stop=(m == GRP - 1),
                    skip_group_check=True)

            # group end: spike bytes = P_t - P_{t-1}, straight to uint8.
            # PSUM->SBUF staging on ACT (DVE can't read 2 PSUM operands).
            if m == GRP - 1:
                W = 2 * (CH + 1)
                ps = o_p.tile([CH, NJP * W], dt.float32, tag="ps")
                for jp in range(NJP):
                    nc.scalar.copy(ps[:, jp * W:(jp + 1) * W], pp_t[jp][:])
                ps4 = ps[:].rearrange("p (q j t) -> p q j t", q=NJP, j=2)
                yk = o_p.tile([CH, GRP * CH], dt.uint8, tag="yk")
                yk4 = yk[:].rearrange("p (q j t) -> p q j t", q=NJP, j=2)
                nc.vector.tensor_tensor(yk4, ps4[:, :, :, 1:CH + 1],
                                        ps4[:, :, :, 0:CH], op=AL.subtract)
                _se = getattr(nc, CFG.get("storeeng", "sync"))
                _se.dma_start(y_pk[b, G], yk[:])


def _make_in_maps(x, v0):
    uv = np.triu(np.ones((CH, CH), dtype=np.float32))
    pksv = np.zeros((GRP, CH, CH), dtype=np.float16)
    for mm in range(GRP):
        for g in range(16):
            for i in range(8):
                pksv[mm, 8 * g + i, 16 * mm + g] = float(1 << i)
    in_maps = []
    for c in range(N_CORES):
        xb = np.ascontiguousarray(x[BC * c:BC * (c + 1)])
        v0b = v0[BC * c:BC * (c + 1)]
        v0tb = np.ascontiguousarray(
            v0b.reshape(BC, NDB, CH).transpose(0, 2, 1).astype(np.float32))
        v0mb = (v0tb - np.float32(0.5)).astype(np.float32)
        in_maps.append({"x": xb, "v0m": v0mb, "u": uv, "pks": pksv})
    return in_maps


def _unpack_output(pk):
    """pk: [B, NGR, CH, GRP*CH] uint8 -> [B, T, D] f32 spike outputs."""
    a = pk.reshape(B, NGR, GRP, 16, NDB, CH)        # b, G, m, g, j, t
    bits = np.unpackbits(a[..., None], axis=-1, bitorder="little")
    # bits: b, G, m, g, j, t, i  ->  b, (G, m, t), (j, g, i)
    out = bits.transpose(0, 1, 2, 5, 4, 3, 6).reshape(B, T, D)
    return out.astype(np.float32) * np.float32(K1000)


def _get_nc():
    if "nc" not in _cache:
        _cache["nc"] = build_nc()
    return _cache["nc"]


def _get_runner():
    """Build (once) a cached jitted SPMD executable over the 8 cores."""
    if "runner" in _cache:
        return _cache["runner"]
    import jax
    from jax.sharding import Mesh, PartitionSpec
    from jax.experimental.shard_map import shard_map
    from concourse import bass2jax

    nc = _get_nc()
    bass2jax.install_neuronx_cc_hook()
    partition_name = nc.partition_id_tensor.name if nc.partition_id_tensor else None
    in_names, out_names, out_avals = [], [], []
    for alloc in nc.m.functions[0].allocations:
        if not isinstance(alloc, mybir.MemoryLocationSet):
            continue
        name = alloc.memorylocations[0].name
        if alloc.kind == "ExternalInput":
            if name != partition_name:
                in_names.append(name)
        elif alloc.kind == "ExternalOutput":
            out_names.append(name)
            out_avals.append(jax.core.ShapedArray(
                tuple(alloc.tensor_shape), dt.np(alloc.dtype)))
    n_params = len(in_names)
    zero_outs = [np.zeros(a.shape, a.dtype) for a in out_avals]
    all_names = in_names + out_names + ([partition_name] if partition_name else [])

    def _body(*args):
        operands = list(args)
        if partition_name is not None:
            operands.append(bass2jax.partition_id_tensor())
        return tuple(bass2jax._bass_exec_p.bind(
            *operands, out_avals=tuple(out_avals), in_names=tuple(all_names),
            out_names=tuple(out_names), lowering_input_output_aliases=(),
            sim_require_finite=True, sim_require_nnan=True, nc=nc))

    devices = jax.devices()[:N_CORES]
    mesh = Mesh(np.asarray(devices), ("core",))
    nin = n_params + len(out_names)
    fn = jax.jit(shard_map(_body, mesh=mesh,
                           in_specs=(PartitionSpec("core"),) * nin,
                           out_specs=(PartitionSpec("core"),) * len(out_names),
                           check_rep=False))
    _cache["runner"] = (fn, in_names, out_names, zero_outs)
    return _cache["runner"]


def kernel(inputs, initial_state):
    import jax
    x = np.ascontiguousarray(np.asarray(inputs, dtype=np.float32))
    v0 = np.ascontiguousarray(np.asarray(initial_state, dtype=np.float32))
    assert x.shape == (B, T, D) and v0.shape == (B, D)
    fn, in_names, out_names, zero_outs = _get_runner()
    in_maps = _make_in_maps(x, v0)
    concat_in = [np.concatenate([np.asarray(in_maps[c][nm])
                                 for c in range(N_CORES)], axis=0)
                 for nm in in_names]
    concat_zero = [np.concatenate([z] * N_CORES, axis=0) for z in zero_outs]
    outs = jax.block_until_ready(fn(*concat_in, *concat_zero))
    pk = np.asarray(outs[out_names.index("y_pk")])
    return _unpack_output(pk)
